# revision 6
# baseline (speedup 1.0000x reference)
"""GNN message-passing kernel for trn2 (8 NeuronCores, SPMD, 4 launches).

Restructured around batched dma_gather (one instruction per 1024-edge chunk
instead of per-edge indirect DMAs) + one-hot matmul segment reduction:

  L1: per-node u = leaky_relu(logmap0(x) @ W_up); stored as hi/lo bf16 pair
      (u = hi + lo accurate to ~2^-18) forming the gather table rows.
  L2: gather u rows per edge (dst-sorted layout), one-hot matmul per
      128-dst block accumulating [sum_hi | sum_lo] in PSUM -> sum_neigh;
      z = sum_neigh @ [W_pl | W_lw_hi]; sel = (relu(z1)-relu(z0) > logit(T));
      sumw = z2; t2 = sel * u (bf16) becomes the next table.
  L3: gather t2 rows, same reduction -> T; s2 = T @ W_lw_lo;
      wsel = sigmoid(s2 + sumw); u3 = wsel * sel * u (bf16 table).
  L4: gather u3 rows, same reduction -> a_x = relu(.);
      out = proj(expmap0(u + a_x)).

Edges are laid out per core as [block-group][quarter][block] segments padded
to 128-edge tiles (K_bq = cross-core max per (block, quarter), so the SPMD
program is identical on all cores; idx/dst-label contents are per-core data).
Quarters arise from dma_gather's int16 index range: the 100352-row table is
addressed as four 25088-row subtables. Host does index preprocessing only.
"""
import os
import sys

sys.path.insert(0, "/opt/trn_rl_repo")

import numpy as np
import ml_dtypes

import concourse.bacc as bacc
import concourse.bass as bass
import concourse.tile as tile
import concourse.mybir as mybir
from concourse import bass_utils, library_config
from concourse.masks import make_identity

F32 = mybir.dt.float32
BF16 = mybir.dt.bfloat16
I16 = mybir.dt.int16
ALU = mybir.AluOpType
ACT = mybir.ActivationFunctionType

NC_N = 8
NSH = 12500               # real nodes per core
NPAD = 12544              # 98 * 128 padded nodes per core
N_ALL = NC_N * NPAD       # 100352
NBLK = 98                 # dst blocks (128 nodes) per core
BGSZ = 7                  # blocks per block-group (<= 8 PSUM banks)
NBG = 14
QN = 4                    # int16 quarters of the table
QROWS = N_ALL // QN       # 25088
CHT = 8                   # max tiles (of 128 edges) per dma_gather chunk
MIN_NORM = 1e-15
ATANH_CLIP = 1.0 - 1e-7
PROJ_MAXN = 1.0 - 4e-3
SEL_THR = float(np.log(np.float64(0.48) / np.float64(0.52)))
BF = ml_dtypes.bfloat16


# ------------------------------------------------------------ static schedule
def make_schedule(K):
    """K: [NBLK][QN] slot counts (multiples of 128, cross-core max).

    Layout order: for bg in 0..6: for q in 0..3: for b in bg-blocks: K[b][q]
    slots.  Returns per-tile block ids, per-block first/last tile, and gather
    chunks (q, tile0, ntiles) each within one quarter.
    """
    tile_blk = []
    chunks = []
    for g in range(NBG):
        for q in range(QN):
            run = []  # tiles of this (bg, q) run
            for b in range(g * BGSZ, (g + 1) * BGSZ):
                nt = K[b][q] // 128
                tile_blk.extend([b] * nt)
                run.extend([b] * nt)
            t0 = len(tile_blk) - len(run)
            n = len(run)
            off = 0
            while off < n:
                c = min(CHT, n - off)
                chunks.append((q, t0 + off, c))
                off += c
    NT = len(tile_blk)
    first = {}
    last = {}
    for t, b in enumerate(tile_blk):
        if b not in first:
            first[b] = t
        last[b] = t
    return dict(tile_blk=tile_blk, chunks=chunks, NT=NT, first=first, last=last)


# ---------------------------------------------------------------- host prep
def host_prep(edge_index):
    src = np.asarray(edge_index[0], dtype=np.int64)
    dst = np.asarray(edge_index[1], dtype=np.int64)
    s_core = src // NSH
    s_row = s_core * NPAD + (src % NSH)
    q_e = s_core // 2
    d_core = dst // NSH
    d_slot = dst % NSH
    b_e = d_slot // 128
    p_e = d_slot % 128

    counts = np.zeros((NC_N, NBLK, QN), np.int64)
    np.add.at(counts, (d_core, b_e, q_e), 1)
    K = (np.ceil(counts.max(axis=0) / 128.0).astype(np.int64) * 128)  # [98, 4]
    Klist = K.tolist()
    sched = make_schedule(Klist)
    NT = sched["NT"]

    # start offset (in slots) of each (b, q) group in the flat layout
    start_bq = np.zeros((NBLK, QN), np.int64)
    off = 0
    for g in range(NBG):
        for q in range(QN):
            for b in range(g * BGSZ, (g + 1) * BGSZ):
                start_bq[b, q] = off
                off += Klist[b][q]
    assert off == NT * 128

    cores = []
    for c in range(NC_N):
        m = d_core == c
        eb, eq = b_e[m], q_e[m]
        eidx = s_row[m] - QROWS * q_e[m]
        elab = p_e[m]
        key = eb * QN + eq
        order = np.argsort(key, kind="stable")
        eb, eq, eidx, elab = eb[order], eq[order], eidx[order], elab[order]

        idx_flat = np.zeros(NT * 128, np.int16)
        lab_flat = np.full(NT * 128, -1.0, np.float32)
        grp_sizes = counts[c].reshape(-1)  # key order = b*4+q
        grp_starts = np.concatenate([[0], np.cumsum(grp_sizes)])
        for b in range(NBLK):
            for q in range(QN):
                gkey = b * QN + q
                s0, s1 = grp_starts[gkey], grp_starts[gkey + 1]
                n = s1 - s0
                if n == 0:
                    continue
                d0 = start_bq[b, q]
                idx_flat[d0:d0 + n] = eidx[s0:s1].astype(np.int16)
                lab_flat[d0:d0 + n] = elab[s0:s1].astype(np.float32)

        idxw = np.empty((128, NT * 8), np.int16)
        for (q, t0, nt) in sched["chunks"]:
            seg = idx_flat[t0 * 128:(t0 + nt) * 128]
            idxw[:, t0 * 8:(t0 + nt) * 8] = np.tile(
                seg.reshape(nt * 8, 16).T, (8, 1))
        dstlab = lab_flat.reshape(NT, 128).T.astype(BF)  # [128, NT]
        cores.append(dict(idxw=idxw, dstlab=dstlab))
    return cores, Klist, sched


# ---------------------------------------------------------------- L1
def build_L1():
    nc = bacc.Bacc("TRN2", target_bir_lowering=False, debug=False,
                   num_devices=NC_N)
    x = nc.dram_tensor("x", [NPAD, 128], F32, kind="ExternalInput").ap()
    Wup = nc.dram_tensor("Wup", [128, 64], F32, kind="ExternalInput").ap()
    tab_o = nc.dram_tensor("tab_o", [NPAD, 128], BF16, kind="ExternalOutput").ap()

    with tile.TileContext(nc) as tc:
        with tc.tile_pool(name="const", bufs=1) as cp, \
             tc.tile_pool(name="sb", bufs=3) as sp, \
             tc.tile_pool(name="ps", bufs=4, space="PSUM") as pp:
            ident = cp.tile([128, 128], F32)
            make_identity(nc, ident[:])
            wu = cp.tile([128, 64], F32)
            nc.sync.dma_start(out=wu[:], in_=Wup[:])
            xa = cp.tile([128, NBLK * 128], F32)
            xa3 = xa[:].rearrange("p (b f) -> p b f", f=128)
            nc.sync.dma_start(out=xa3,
                              in_=x.rearrange("(b p) f -> p b f", p=128))
            n2_all = cp.tile([128, NBLK], F32)
            xT_all = cp.tile([128, NBLK * 128], F32)
            xT3 = xT_all[:].rearrange("p (b f) -> p b f", f=128)
            hl_all = cp.tile([128, NBLK * 128], BF16)
            hl3 = hl_all[:].rearrange("p (b f) -> p b f", f=128)

            for b in range(NBLK):
                sq = sp.tile([128, 128], F32, tag="sq")
                nc.scalar.activation(out=sq[:], in_=xa3[:, b, :],
                                     func=ACT.Square,
                                     accum_out=n2_all[:, b:b + 1])
            # scale chain on [128, 98]
            nv = cp.tile([128, NBLK], F32)
            nc.scalar.activation(out=nv[:], in_=n2_all[:], func=ACT.Sqrt)
            nm = cp.tile([128, NBLK], F32)
            nc.vector.tensor_scalar_max(nm[:], nv[:], MIN_NORM)
            cl = cp.tile([128, NBLK], F32)
            nc.vector.tensor_scalar_min(cl[:], nm[:], ATANH_CLIP)
            num = cp.tile([128, NBLK], F32)
            nc.vector.tensor_scalar_add(num[:], cl[:], 1.0)
            den = cp.tile([128, NBLK], F32)
            nc.vector.tensor_scalar(out=den[:], in0=cl[:], scalar1=-1.0,
                                    scalar2=1.0, op0=ALU.mult, op1=ALU.add)
            rden = cp.tile([128, NBLK], F32)
            nc.vector.reciprocal(rden[:], den[:])
            qt = cp.tile([128, NBLK], F32)
            nc.vector.tensor_tensor(out=qt[:], in0=num[:], in1=rden[:],
                                    op=ALU.mult)
            lq = cp.tile([128, NBLK], F32)
            nc.scalar.activation(out=lq[:], in_=qt[:], func=ACT.Ln)
            rnm = cp.tile([128, NBLK], F32)
            nc.vector.reciprocal(rnm[:], nm[:])
            sc1 = cp.tile([128, NBLK], F32)
            nc.vector.tensor_tensor(out=sc1[:], in0=lq[:], in1=rnm[:],
                                    op=ALU.mult)
            s_all = cp.tile([128, NBLK], F32)
            nc.vector.tensor_scalar_mul(s_all[:], sc1[:], 0.5)

            for b in range(NBLK):
                xtan = sp.tile([128, 128], F32, tag="xtan")
                nc.vector.tensor_tensor(
                    out=xtan[:], in0=xa3[:, b, :],
                    in1=s_all[:, b:b + 1].to_broadcast([128, 128]),
                    op=ALU.mult)
                psT = pp.tile([128, 128], F32, tag="psT", space="PSUM")
                nc.tensor.transpose(psT[:], xtan[:], ident[:])
                nc.vector.tensor_copy(out=xT3[:, b, :], in_=psT[:])
                psU = pp.tile([128, 64], F32, tag="psU", space="PSUM")
                nc.tensor.matmul(psU[:], lhsT=xT3[:, b, :], rhs=wu[:],
                                 start=True, stop=True)
                tmp = sp.tile([128, 64], F32, tag="tmp")
                nc.vector.tensor_scalar_mul(tmp[:], psU[:], 0.01)
                u_sb = sp.tile([128, 64], F32, tag="u")
                nc.vector.tensor_max(u_sb[:], psU[:], tmp[:])
                nc.vector.tensor_copy(out=hl3[:, b, 0:64], in_=u_sb[:])
                hif = sp.tile([128, 64], F32, tag="hif")
                nc.vector.tensor_copy(out=hif[:], in_=hl3[:, b, 0:64])
                nc.vector.tensor_tensor(out=hl3[:, b, 64:128], in0=u_sb[:],
                                        in1=hif[:], op=ALU.subtract)
            nc.sync.dma_start(
                out=tab_o.rearrange("(b p) f -> p b f", p=128),
                in_=hl3)
    nc.compile()
    return nc


# ------------------------------------------------- shared gather-reduce loop
def emit_gather_reduce(nc, tc, sched, tab, idx_t, dst_t, iota_t, gp, Sp, pp,
                       rhs_cols, ps_tag, on_block):
    """Main loop: chunked dma_gather + one-hot matmul reduction.

    on_block(b, ps) is called right after block b's last accumulating matmul.
    """
    tile_blk = sched["tile_blk"]
    chunks = sched["chunks"]
    first, last = sched["first"], sched["last"]
    OB = 8

    ps_tiles = {}
    # S tiles built in batches of OB aligned to global tile index
    S_cur = {"tile": None, "base": -1}

    def get_S(t):
        base = (t // OB) * OB
        if S_cur["base"] != base:
            n = min(OB, sched["NT"] - base)
            S = Sp.tile([128, OB * 128], BF16, tag="S")
            nc.vector.tensor_tensor(
                out=S[:].rearrange("p (k d) -> p k d", d=128)[:, 0:n, :],
                in0=dst_t[:, base:base + n].to_broadcast([128, n, 128]),
                in1=iota_t[:].unsqueeze(1).broadcast_to([128, n, 128]),
                op=ALU.is_equal)
            S_cur["tile"] = S
            S_cur["base"] = base
        return S_cur["tile"][:, (t - S_cur["base"]) * 128:
                             (t - S_cur["base"] + 1) * 128]

    for (q, t0, ntl) in chunks:
        ch = ntl * 128
        gt = gp.tile([128, CHT * 128], BF16, tag="gt")
        gt3 = gt[:].rearrange("p (k f) -> p k f", f=128)
        nc.gpsimd.dma_gather(
            gt3[:, 0:ntl, :], tab[QROWS * q:QROWS * (q + 1), :],
            idx_t[:, t0 * 8:(t0 + ntl) * 8], ch, ch, 128)
        for tl in range(ntl):
            t = t0 + tl
            b = tile_blk[t]
            if b not in ps_tiles:
                ps_tiles[b] = pp.tile([128, rhs_cols], F32,
                                      tag=f"{ps_tag}{b % BGSZ}", space="PSUM",
                                      name=f"{ps_tag}{b % BGSZ}")
            S = get_S(t)
            nc.tensor.matmul(ps_tiles[b][:], lhsT=S,
                             rhs=gt3[:, tl, 0:rhs_cols],
                             start=(t == first[b]), stop=(t == last[b]))
            if t == last[b]:
                on_block(b, ps_tiles.pop(b))


# ---------------------------------------------------------------- L2
def build_L2(sched):
    nc = bacc.Bacc("TRN2", target_bir_lowering=False, debug=False,
                   num_devices=NC_N)
    NT = sched["NT"]
    tab = nc.dram_tensor("tab", [N_ALL, 128], BF16, kind="ExternalInput").ap()
    idxs = nc.dram_tensor("idxs", [128, NT * 8], I16, kind="ExternalInput").ap()
    dlab = nc.dram_tensor("dlab", [128, NT], BF16, kind="ExternalInput").ap()
    iota = nc.dram_tensor("iota", [128, 128], BF16, kind="ExternalInput").ap()
    wc_rep = nc.dram_tensor("wc_rep", [128, 3 * 64], F32,
                            kind="ExternalInput").ap()
    u_own = nc.dram_tensor("u_own", [NPAD, 128], BF16, kind="ExternalInput").ap()
    sel_o = nc.dram_tensor("sel_o", [128, NBLK], F32, kind="ExternalOutput").ap()
    sumw_o = nc.dram_tensor("sumw_o", [128, NBLK], F32,
                            kind="ExternalOutput").ap()
    t2_o = nc.dram_tensor("t2_o", [NPAD, 64], BF16, kind="ExternalOutput").ap()

    with tile.TileContext(nc) as tc:
        with tc.tile_pool(name="const", bufs=1) as cp, \
             tc.tile_pool(name="sb", bufs=3) as sp, \
             tc.tile_pool(name="g", bufs=3) as gp, \
             tc.tile_pool(name="S", bufs=2) as Sp, \
             tc.tile_pool(name="ps", bufs=1, space="PSUM") as pp:
            nc.gpsimd.load_library(library_config.mlp)
            iota_t = cp.tile([128, 128], BF16)
            nc.sync.dma_start(out=iota_t[:], in_=iota[:])
            wc_t = cp.tile([128, 3 * 64], F32)
            nc.sync.dma_start(out=wc_t[:], in_=wc_rep[:])
            wc3 = wc_t[:].rearrange("p (j f) -> p j f", f=64)
            dst_t = cp.tile([128, NT], BF16)
            nc.sync.dma_start(out=dst_t[:], in_=dlab[:])
            idx_t = cp.tile([128, NT * 8], I16)
            nc.sync.dma_start(out=idx_t[:], in_=idxs[:])
            uo = cp.tile([128, NBLK * 128], BF16)
            uo3 = uo[:].rearrange("p (b f) -> p b f", f=128)
            nc.sync.dma_start(out=uo3,
                              in_=u_own.rearrange("(b p) f -> p b f", p=128))
            z_all = cp.tile([128, NBLK * 3], F32)
            z3 = z_all[:].rearrange("p (b j) -> p b j", j=3)

            def on_block(b, ps):
                rh = sp.tile([128, 64], F32, tag="rh")
                nc.scalar.copy(out=rh[:], in_=ps[:, 0:64])
                res = sp.tile([128, 64], F32, tag="res")
                nc.vector.tensor_add(res[:], rh[:], ps[:, 64:128])
                prod = sp.tile([128, 3 * 64], F32, tag="prod")
                nc.vector.tensor_tensor(
                    out=prod[:].rearrange("p (j f) -> p j f", f=64),
                    in0=res[:].unsqueeze(1).broadcast_to([128, 3, 64]),
                    in1=wc3, op=ALU.mult)
                nc.vector.tensor_reduce(
                    out=z3[:, b, :],
                    in_=prod[:].rearrange("p (j f) -> p j f", f=64),
                    axis=mybir.AxisListType.X, op=ALU.add)

            emit_gather_reduce(nc, tc, sched, tab, idx_t, dst_t, iota_t,
                               gp, Sp, pp, 128, "ps", on_block)

            # epilogue: sel / sumw / t2
            zr = cp.tile([128, NBLK * 2], F32)
            zr3 = zr[:].rearrange("p (b j) -> p b j", j=2)
            nc.vector.tensor_scalar_max(zr3[:, :, :], z3[:, :, 0:2], 0.0)
            dd = cp.tile([128, NBLK], F32)
            nc.vector.tensor_tensor(out=dd[:], in0=zr3[:, :, 1],
                                    in1=zr3[:, :, 0], op=ALU.subtract)
            sel = cp.tile([128, NBLK], F32)
            nc.vector.tensor_scalar(out=sel[:], in0=dd[:], scalar1=SEL_THR,
                                    scalar2=0.0, op0=ALU.is_gt)
            nc.sync.dma_start(out=sel_o[:], in_=sel[:])
            sumw = cp.tile([128, NBLK], F32)
            nc.vector.tensor_copy(out=sumw[:], in_=z3[:, :, 2])
            nc.sync.dma_start(out=sumw_o[:], in_=sumw[:])
            t2_all = cp.tile([128, NBLK * 64], BF16)
            t23 = t2_all[:].rearrange("p (b f) -> p b f", f=64)
            for b in range(NBLK):
                ub = sp.tile([128, 64], F32, tag="ub")
                nc.vector.tensor_add(ub[:], uo3[:, b, 0:64], uo3[:, b, 64:128])
                nc.vector.tensor_tensor(
                    out=t23[:, b, :], in0=ub[:],
                    in1=sel[:, b:b + 1].to_broadcast([128, 64]), op=ALU.mult)
            nc.sync.dma_start(
                out=t2_o.rearrange("(b p) f -> p b f", p=128),
                in_=t23)
    nc.compile()
    return nc


# ---------------------------------------------------------------- L3
def build_L3(sched):
    nc = bacc.Bacc("TRN2", target_bir_lowering=False, debug=False,
                   num_devices=NC_N)
    NT = sched["NT"]
    tab = nc.dram_tensor("tab", [N_ALL, 128], BF16, kind="ExternalInput").ap()
    idxs = nc.dram_tensor("idxs", [128, NT * 8], I16, kind="ExternalInput").ap()
    dlab = nc.dram_tensor("dlab", [128, NT], BF16, kind="ExternalInput").ap()
    iota = nc.dram_tensor("iota", [128, 128], BF16, kind="ExternalInput").ap()
    wl_rep = nc.dram_tensor("wl_rep", [128, 64], F32, kind="ExternalInput").ap()
    u_own = nc.dram_tensor("u_own", [NPAD, 128], BF16, kind="ExternalInput").ap()
    sel_i = nc.dram_tensor("sel_i", [128, NBLK], F32, kind="ExternalInput").ap()
    sumw_i = nc.dram_tensor("sumw_i", [128, NBLK], F32,
                            kind="ExternalInput").ap()
    u3_o = nc.dram_tensor("u3_o", [NPAD, 64], BF16, kind="ExternalOutput").ap()

    with tile.TileContext(nc) as tc:
        with tc.tile_pool(name="const", bufs=1) as cp, \
             tc.tile_pool(name="sb", bufs=3) as sp, \
             tc.tile_pool(name="g", bufs=3) as gp, \
             tc.tile_pool(name="S", bufs=2) as Sp, \
             tc.tile_pool(name="ps", bufs=1, space="PSUM") as pp:
            nc.gpsimd.load_library(library_config.mlp)
            iota_t = cp.tile([128, 128], BF16)
            nc.sync.dma_start(out=iota_t[:], in_=iota[:])
            wl_t = cp.tile([128, 64], F32)
            nc.sync.dma_start(out=wl_t[:], in_=wl_rep[:])
            dst_t = cp.tile([128, NT], BF16)
            nc.sync.dma_start(out=dst_t[:], in_=dlab[:])
            idx_t = cp.tile([128, NT * 8], I16)
            nc.sync.dma_start(out=idx_t[:], in_=idxs[:])
            uo = cp.tile([128, NBLK * 128], BF16)
            uo3 = uo[:].rearrange("p (b f) -> p b f", f=128)
            nc.sync.dma_start(out=uo3,
                              in_=u_own.rearrange("(b p) f -> p b f", p=128))
            sel_t = cp.tile([128, NBLK], F32)
            nc.sync.dma_start(out=sel_t[:], in_=sel_i[:])
            sumw_t = cp.tile([128, NBLK], F32)
            nc.sync.dma_start(out=sumw_t[:], in_=sumw_i[:])
            s2_all = cp.tile([128, NBLK], F32)

            def on_block(b, ps):
                prod = sp.tile([128, 64], F32, tag="prod")
                nc.vector.tensor_tensor(out=prod[:], in0=ps[:], in1=wl_t[:],
                                        op=ALU.mult)
                nc.vector.tensor_reduce(
                    out=s2_all[:, b:b + 1], in_=prod[:],
                    axis=mybir.AxisListType.X, op=ALU.add)

            emit_gather_reduce(nc, tc, sched, tab, idx_t, dst_t, iota_t,
                               gp, Sp, pp, 64, "ps", on_block)

            zs = cp.tile([128, NBLK], F32)
            nc.vector.tensor_add(zs[:], s2_all[:], sumw_t[:])
            wsel = cp.tile([128, NBLK], F32)
            nc.scalar.activation(out=wsel[:], in_=zs[:], func=ACT.Sigmoid)
            g_all = cp.tile([128, NBLK], F32)
            nc.vector.tensor_tensor(out=g_all[:], in0=wsel[:], in1=sel_t[:],
                                    op=ALU.mult)
            u3_all = cp.tile([128, NBLK * 64], BF16)
            u33 = u3_all[:].rearrange("p (b f) -> p b f", f=64)
            for b in range(NBLK):
                ub = sp.tile([128, 64], F32, tag="ub")
                nc.vector.tensor_add(ub[:], uo3[:, b, 0:64], uo3[:, b, 64:128])
                nc.vector.tensor_tensor(
                    out=u33[:, b, :], in0=ub[:],
                    in1=g_all[:, b:b + 1].to_broadcast([128, 64]), op=ALU.mult)
            nc.sync.dma_start(
                out=u3_o.rearrange("(b p) f -> p b f", p=128),
                in_=u33)
    nc.compile()
    return nc


# ---------------------------------------------------------------- L4
def build_L4(sched):
    nc = bacc.Bacc("TRN2", target_bir_lowering=False, debug=False,
                   num_devices=NC_N)
    NT = sched["NT"]
    tab = nc.dram_tensor("tab", [N_ALL, 128], BF16, kind="ExternalInput").ap()
    idxs = nc.dram_tensor("idxs", [128, NT * 8], I16, kind="ExternalInput").ap()
    dlab = nc.dram_tensor("dlab", [128, NT], BF16, kind="ExternalInput").ap()
    iota = nc.dram_tensor("iota", [128, 128], BF16, kind="ExternalInput").ap()
    u_own = nc.dram_tensor("u_own", [NPAD, 128], BF16, kind="ExternalInput").ap()
    out_o = nc.dram_tensor("out_o", [NPAD, 64], F32, kind="ExternalOutput").ap()

    with tile.TileContext(nc) as tc:
        with tc.tile_pool(name="const", bufs=1) as cp, \
             tc.tile_pool(name="sb", bufs=3) as sp, \
             tc.tile_pool(name="g", bufs=3) as gp, \
             tc.tile_pool(name="S", bufs=2) as Sp, \
             tc.tile_pool(name="ps", bufs=1, space="PSUM") as pp:
            nc.gpsimd.load_library(library_config.mlp)
            iota_t = cp.tile([128, 128], BF16)
            nc.sync.dma_start(out=iota_t[:], in_=iota[:])
            dst_t = cp.tile([128, NT], BF16)
            nc.sync.dma_start(out=dst_t[:], in_=dlab[:])
            idx_t = cp.tile([128, NT * 8], I16)
            nc.sync.dma_start(out=idx_t[:], in_=idxs[:])
            uo = cp.tile([128, NBLK * 128], BF16)
            uo3 = uo[:].rearrange("p (b f) -> p b f", f=128)
            nc.sync.dma_start(out=uo3,
                              in_=u_own.rearrange("(b p) f -> p b f", p=128))
            o_all = cp.tile([128, NBLK * 64], F32)
            o3 = o_all[:].rearrange("p (b f) -> p b f", f=64)
            n2_all = cp.tile([128, NBLK], F32)

            def on_block(b, ps):
                ax = sp.tile([128, 64], F32, tag="ax")
                nc.vector.tensor_scalar_max(ax[:], ps[:], 0.0)
                ub = sp.tile([128, 64], F32, tag="ub")
                nc.vector.tensor_add(ub[:], uo3[:, b, 0:64], uo3[:, b, 64:128])
                nc.vector.tensor_add(o3[:, b, :], ub[:], ax[:])
                sq = sp.tile([128, 64], F32, tag="sq")
                nc.scalar.activation(out=sq[:], in_=o3[:, b, :],
                                     func=ACT.Square,
                                     accum_out=n2_all[:, b:b + 1])

            emit_gather_reduce(nc, tc, sched, tab, idx_t, dst_t, iota_t,
                               gp, Sp, pp, 64, "ps", on_block)

            # expmap0 + proj factors
            nv = cp.tile([128, NBLK], F32)
            nc.scalar.activation(out=nv[:], in_=n2_all[:], func=ACT.Sqrt)
            nm = cp.tile([128, NBLK], F32)
            nc.vector.tensor_scalar_max(nm[:], nv[:], MIN_NORM)
            th = cp.tile([128, NBLK], F32)
            nc.scalar.activation(out=th[:], in_=nm[:], func=ACT.Tanh)
            rnm = cp.tile([128, NBLK], F32)
            nc.vector.reciprocal(rnm[:], nm[:])
            f1 = cp.tile([128, NBLK], F32)
            nc.vector.tensor_tensor(out=f1[:], in0=th[:], in1=rnm[:],
                                    op=ALU.mult)
            rt = cp.tile([128, NBLK], F32)
            nc.vector.reciprocal(rt[:], th[:])
            cap = cp.tile([128, NBLK], F32)
            nc.vector.tensor_scalar(out=cap[:], in0=rt[:], scalar1=PROJ_MAXN,
                                    scalar2=1.0, op0=ALU.mult, op1=ALU.min)
            f2 = cp.tile([128, NBLK], F32)
            nc.vector.tensor_tensor(out=f2[:], in0=f1[:], in1=cap[:],
                                    op=ALU.mult)
            oo_all = cp.tile([128, NBLK * 64], F32)
            oo3 = oo_all[:].rearrange("p (b f) -> p b f", f=64)
            for b in range(NBLK):
                nc.vector.tensor_tensor(
                    out=oo3[:, b, :], in0=o3[:, b, :],
                    in1=f2[:, b:b + 1].to_broadcast([128, 64]), op=ALU.mult)
            nc.sync.dma_start(
                out=out_o.rearrange("(b p) f -> p b f", p=128),
                in_=oo3)
    nc.compile()
    return nc


# ---------------------------------------------------------------- runner
def _run(nc, in_maps, trace):
    return bass_utils.run_bass_kernel_spmd(
        nc, in_maps, core_ids=list(range(NC_N)), trace=trace)


def kernel(x, edge_index, W_up, W_pl, W_lw, trace=None):
    if trace is None:
        trace = bool(int(os.environ.get("GNN_TRACE", "0")))

    x = np.asarray(x, np.float32)
    W_up = np.asarray(W_up, np.float32)
    W_pl = np.asarray(W_pl, np.float32)
    W_lw = np.asarray(W_lw, np.float32)
    cores, Klist, sched = host_prep(edge_index)
    exec_times = []
    kernel.last_sched = sched

    iota = np.tile(np.arange(128, dtype=np.float32)[None, :],
                   (128, 1)).astype(BF)
    Wcat = np.concatenate([W_pl, W_lw[64:128]], axis=1)        # [64, 3]
    wc_rep = np.tile(Wcat.T.reshape(1, 3 * 64), (128, 1)).astype(np.float32)
    wl_rep = np.tile(W_lw[0:64, 0].reshape(1, 64), (128, 1)).astype(np.float32)

    # ---- L1
    x_pad = np.zeros((NC_N, NPAD, 128), np.float32)
    for c in range(NC_N):
        x_pad[c, :NSH] = x[c * NSH:(c + 1) * NSH]
    nc1 = build_L1()
    r1 = _run(nc1, [{"x": x_pad[c], "Wup": W_up} for c in range(NC_N)], trace)
    exec_times.append(r1.exec_time_ns)
    shards = [r1.results[c]["tab_o"] for c in range(NC_N)]

    # ---- L2
    tab2 = np.concatenate(shards, axis=0)                      # [N_ALL, 128]
    nc2 = build_L2(sched)
    r2 = _run(nc2, [{"tab": tab2, "idxs": cores[c]["idxw"],
                     "dlab": cores[c]["dstlab"], "iota": iota,
                     "wc_rep": wc_rep, "u_own": shards[c]}
                    for c in range(NC_N)], trace)
    exec_times.append(r2.exec_time_ns)
    sel = [r2.results[c]["sel_o"] for c in range(NC_N)]
    sumw = [r2.results[c]["sumw_o"] for c in range(NC_N)]
    t2 = [r2.results[c]["t2_o"] for c in range(NC_N)]

    # ---- L3
    tab3 = np.zeros((N_ALL, 128), BF)
    tab3[:, 0:64] = np.concatenate(t2, axis=0)
    nc3 = build_L3(sched)
    r3 = _run(nc3, [{"tab": tab3, "idxs": cores[c]["idxw"],
                     "dlab": cores[c]["dstlab"], "iota": iota,
                     "wl_rep": wl_rep, "u_own": shards[c],
                     "sel_i": sel[c], "sumw_i": sumw[c]}
                    for c in range(NC_N)], trace)
    exec_times.append(r3.exec_time_ns)
    u3 = [r3.results[c]["u3_o"] for c in range(NC_N)]

    # ---- L4
    tab4 = np.zeros((N_ALL, 128), BF)
    tab4[:, 0:64] = np.concatenate(u3, axis=0)
    nc4 = build_L4(sched)
    r4 = _run(nc4, [{"tab": tab4, "idxs": cores[c]["idxw"],
                     "dlab": cores[c]["dstlab"], "iota": iota,
                     "u_own": shards[c]}
                    for c in range(NC_N)], trace)
    exec_times.append(r4.exec_time_ns)
    out = np.concatenate([r4.results[c]["out_o"][:NSH] for c in range(NC_N)],
                         axis=0)

    kernel.last_exec_times = exec_times
    return out


# revision 11
# speedup vs baseline: 1.4060x; 1.4060x over previous
"""GNN message-passing kernel for trn2 (8 NeuronCores, SPMD, 4 launches).

Restructured around batched dma_gather (one instruction per 1024-edge chunk
instead of per-edge indirect DMAs) + one-hot matmul segment reduction:

  L1: per-node u = leaky_relu(logmap0(x) @ W_up); stored as hi/lo bf16 pair
      (u = hi + lo accurate to ~2^-18) forming the gather table rows.
  L2: gather u rows per edge (dst-sorted layout), one-hot matmul per
      128-dst block accumulating [sum_hi | sum_lo] in PSUM -> sum_neigh;
      z = sum_neigh @ [W_pl | W_lw_hi]; sel = (relu(z1)-relu(z0) > logit(T));
      sumw = z2; t2 = sel * u (bf16) becomes the next table.
  L3: gather t2 rows, same reduction -> T; s2 = T @ W_lw_lo;
      wsel = sigmoid(s2 + sumw); u3 = wsel * sel * u (bf16 table).
  L4: gather u3 rows, same reduction -> a_x = relu(.);
      out = proj(expmap0(u + a_x)).

Edges are laid out per core as [block-group][quarter][block] segments padded
to 128-edge tiles (K_bq = cross-core max per (block, quarter), so the SPMD
program is identical on all cores; idx/dst-label contents are per-core data).
Quarters arise from dma_gather's int16 index range: the 100352-row table is
addressed as four 25088-row subtables. Host does index preprocessing only.
"""
import os
import sys

sys.path.insert(0, "/opt/trn_rl_repo")

import numpy as np
import ml_dtypes

import concourse.bacc as bacc
import concourse.bass as bass
import concourse.tile as tile
import concourse.mybir as mybir
from concourse import bass_utils, library_config
from concourse.masks import make_identity

F32 = mybir.dt.float32
BF16 = mybir.dt.bfloat16
I16 = mybir.dt.int16
ALU = mybir.AluOpType
ACT = mybir.ActivationFunctionType

NC_N = 8
NSH = 12500               # real nodes per core
NPAD = 12544              # 98 * 128 padded nodes per core
N_ALL = NC_N * NPAD       # 100352
NBLK = 98                 # dst blocks (128 nodes) per core
BGSZ = 7                  # blocks per block-group (<= 8 PSUM banks)
NBG = 14
QN = 4                    # int16 quarters of the table
QROWS = N_ALL // QN       # 25088
CHT = 8                   # max tiles (of 128 edges) per dma_gather chunk
MIN_NORM = 1e-15
ATANH_CLIP = 1.0 - 1e-7
PROJ_MAXN = 1.0 - 4e-3
SEL_THR = float(np.log(np.float64(0.48) / np.float64(0.52)))
BF = ml_dtypes.bfloat16


# ------------------------------------------------------------ static schedule
def make_schedule(K):
    """K: [NBLK][QN] slot counts (multiples of 128, cross-core max).

    Layout order: for bg in 0..6: for q in 0..3: for b in bg-blocks: K[b][q]
    slots.  Returns per-tile block ids, per-block first/last tile, and gather
    chunks (q, tile0, ntiles) each within one quarter.
    """
    tile_blk = []
    chunks = []
    for g in range(NBG):
        for q in range(QN):
            run = []  # tiles of this (bg, q) run
            for b in range(g * BGSZ, (g + 1) * BGSZ):
                nt = K[b][q] // 128
                tile_blk.extend([b] * nt)
                run.extend([b] * nt)
            t0 = len(tile_blk) - len(run)
            n = len(run)
            off = 0
            while off < n:
                c = min(CHT, n - off)
                chunks.append((q, t0 + off, c))
                off += c
    NT = len(tile_blk)
    first = {}
    last = {}
    for t, b in enumerate(tile_blk):
        if b not in first:
            first[b] = t
        last[b] = t
    return dict(tile_blk=tile_blk, chunks=chunks, NT=NT, first=first, last=last)


# ---------------------------------------------------------------- host prep
def _balance_slots(deg):
    """deg: [NPAD, QN] per-node in-degree by src-quarter for one core.

    Assign nodes to 98 blocks of 128 slots so per-(block, quarter) loads
    stay <= CAP for blocks 1..97; block 0 absorbs overflow.  Returns
    slot_of[node_local] in [0, NPAD).
    """
    # weighted LPT: blocks 0-1 target 640 (overflow, same ids on all cores),
    # the rest target 512; place each node to minimize the worst
    # load-to-target ratio among quarters.
    target = np.full(NBLK, 500.0)
    target[0:2] = 900.0
    order = np.argsort(-deg.sum(axis=1), kind="stable")
    L = np.zeros((NBLK, QN), np.float64)
    filled = np.zeros(NBLK, np.int64)
    blk_of = np.empty(NPAD, np.int64)
    for n in order:
        d = deg[n]
        cand = np.flatnonzero(filled < 128)
        ratio = ((L[cand] + d[None, :]).max(axis=1)) / target[cand]
        b = cand[np.argmin(ratio)]
        blk_of[n] = b
        L[b] += d
        filled[b] += 1
    slot_of = np.empty(NPAD, np.int64)
    nxt = np.zeros(NBLK, np.int64)
    for n in range(NPAD):
        b = blk_of[n]
        slot_of[n] = b * 128 + nxt[b]
        nxt[b] += 1
    return slot_of


def host_prep(edge_index):
    src = np.asarray(edge_index[0], dtype=np.int64)
    dst = np.asarray(edge_index[1], dtype=np.int64)
    s_core = src // NSH
    q_e = s_core // 2
    d_core = dst // NSH

    # per-core balanced node->slot permutation (slot = block*128 + lane)
    deg = np.zeros((NC_N, NPAD, QN), np.int64)
    np.add.at(deg, (d_core, dst % NSH, q_e), 1)
    slot_of = np.stack([_balance_slots(deg[c]) for c in range(NC_N)])  # [8, NPAD]

    s_row = s_core * NPAD + slot_of[s_core, src % NSH]
    d_slot = slot_of[d_core, dst % NSH]
    b_e = d_slot // 128
    p_e = d_slot % 128

    counts = np.zeros((NC_N, NBLK, QN), np.int64)
    np.add.at(counts, (d_core, b_e, q_e), 1)
    K = (np.ceil(counts.max(axis=0) / 128.0).astype(np.int64) * 128)  # [98, 4]
    Klist = K.tolist()
    sched = make_schedule(Klist)
    NT = sched["NT"]

    # start offset (in slots) of each (b, q) group in the flat layout
    start_bq = np.zeros((NBLK, QN), np.int64)
    off = 0
    for g in range(NBG):
        for q in range(QN):
            for b in range(g * BGSZ, (g + 1) * BGSZ):
                start_bq[b, q] = off
                off += Klist[b][q]
    assert off == NT * 128

    cores = []
    for c in range(NC_N):
        m = d_core == c
        eb, eq = b_e[m], q_e[m]
        eidx = s_row[m] - QROWS * q_e[m]
        elab = p_e[m]
        key = eb * QN + eq
        order = np.argsort(key, kind="stable")
        eb, eq, eidx, elab = eb[order], eq[order], eidx[order], elab[order]

        idx_flat = np.zeros(NT * 128, np.int16)
        lab_flat = np.full(NT * 128, -1.0, np.float32)
        grp_sizes = counts[c].reshape(-1)  # key order = b*4+q
        grp_starts = np.concatenate([[0], np.cumsum(grp_sizes)])
        for b in range(NBLK):
            for q in range(QN):
                gkey = b * QN + q
                s0, s1 = grp_starts[gkey], grp_starts[gkey + 1]
                n = s1 - s0
                if n == 0:
                    continue
                d0 = start_bq[b, q]
                idx_flat[d0:d0 + n] = eidx[s0:s1].astype(np.int16)
                lab_flat[d0:d0 + n] = elab[s0:s1].astype(np.float32)

        idxw = np.empty((128, NT * 8), np.int16)
        for (q, t0, nt) in sched["chunks"]:
            seg = idx_flat[t0 * 128:(t0 + nt) * 128]
            idxw[:, t0 * 8:(t0 + nt) * 8] = np.tile(
                seg.reshape(nt * 8, 16).T, (8, 1))
        dstlab = lab_flat.reshape(NT, 128).T.astype(BF)  # [128, NT]
        cores.append(dict(idxw=idxw, dstlab=dstlab))
    return cores, Klist, sched, slot_of


# ---------------------------------------------------------------- L1
def build_L1():
    nc = bacc.Bacc("TRN2", target_bir_lowering=False, debug=False,
                   num_devices=NC_N)
    x = nc.dram_tensor("x", [NPAD, 128], F32, kind="ExternalInput").ap()
    Wup = nc.dram_tensor("Wup", [128, 64], F32, kind="ExternalInput").ap()
    tab_o = nc.dram_tensor("tab_o", [NPAD, 128], BF16, kind="ExternalOutput").ap()

    with tile.TileContext(nc) as tc:
        with tc.tile_pool(name="const", bufs=1) as cp, \
             tc.tile_pool(name="sb", bufs=3) as sp, \
             tc.tile_pool(name="ps", bufs=4, space="PSUM") as pp:
            ident = cp.tile([128, 128], F32)
            make_identity(nc, ident[:])
            wu = cp.tile([128, 64], F32)
            nc.sync.dma_start(out=wu[:], in_=Wup[:])
            xa = cp.tile([128, NBLK * 128], F32)
            xa3 = xa[:].rearrange("p (b f) -> p b f", f=128)
            nc.sync.dma_start(out=xa3,
                              in_=x.rearrange("(b p) f -> p b f", p=128))
            n2_all = cp.tile([128, NBLK], F32)
            xT_all = cp.tile([128, NBLK * 128], F32)
            xT3 = xT_all[:].rearrange("p (b f) -> p b f", f=128)
            hl_all = cp.tile([128, NBLK * 128], BF16)
            hl3 = hl_all[:].rearrange("p (b f) -> p b f", f=128)

            for b in range(NBLK):
                sq = sp.tile([128, 128], F32, tag="sq")
                nc.scalar.activation(out=sq[:], in_=xa3[:, b, :],
                                     func=ACT.Square,
                                     accum_out=n2_all[:, b:b + 1])
            # scale chain on [128, 98]
            nv = cp.tile([128, NBLK], F32)
            nc.scalar.activation(out=nv[:], in_=n2_all[:], func=ACT.Sqrt)
            nm = cp.tile([128, NBLK], F32)
            nc.vector.tensor_scalar_max(nm[:], nv[:], MIN_NORM)
            cl = cp.tile([128, NBLK], F32)
            nc.vector.tensor_scalar_min(cl[:], nm[:], ATANH_CLIP)
            num = cp.tile([128, NBLK], F32)
            nc.vector.tensor_scalar_add(num[:], cl[:], 1.0)
            den = cp.tile([128, NBLK], F32)
            nc.vector.tensor_scalar(out=den[:], in0=cl[:], scalar1=-1.0,
                                    scalar2=1.0, op0=ALU.mult, op1=ALU.add)
            rden = cp.tile([128, NBLK], F32)
            nc.vector.reciprocal(rden[:], den[:])
            qt = cp.tile([128, NBLK], F32)
            nc.vector.tensor_tensor(out=qt[:], in0=num[:], in1=rden[:],
                                    op=ALU.mult)
            lq = cp.tile([128, NBLK], F32)
            nc.scalar.activation(out=lq[:], in_=qt[:], func=ACT.Ln)
            rnm = cp.tile([128, NBLK], F32)
            nc.vector.reciprocal(rnm[:], nm[:])
            sc1 = cp.tile([128, NBLK], F32)
            nc.vector.tensor_tensor(out=sc1[:], in0=lq[:], in1=rnm[:],
                                    op=ALU.mult)
            s_all = cp.tile([128, NBLK], F32)
            nc.vector.tensor_scalar_mul(s_all[:], sc1[:], 0.5)

            for b in range(NBLK):
                xtan = sp.tile([128, 128], F32, tag="xtan")
                nc.vector.tensor_tensor(
                    out=xtan[:], in0=xa3[:, b, :],
                    in1=s_all[:, b:b + 1].to_broadcast([128, 128]),
                    op=ALU.mult)
                psT = pp.tile([128, 128], F32, tag="psT", space="PSUM")
                nc.tensor.transpose(psT[:], xtan[:], ident[:])
                nc.vector.tensor_copy(out=xT3[:, b, :], in_=psT[:])
                psU = pp.tile([128, 64], F32, tag="psU", space="PSUM")
                nc.tensor.matmul(psU[:], lhsT=xT3[:, b, :], rhs=wu[:],
                                 start=True, stop=True)
                tmp = sp.tile([128, 64], F32, tag="tmp")
                nc.vector.tensor_scalar_mul(tmp[:], psU[:], 0.01)
                u_sb = sp.tile([128, 64], F32, tag="u")
                nc.vector.tensor_max(u_sb[:], psU[:], tmp[:])
                nc.vector.tensor_copy(out=hl3[:, b, 0:64], in_=u_sb[:])
                hif = sp.tile([128, 64], F32, tag="hif")
                nc.vector.tensor_copy(out=hif[:], in_=hl3[:, b, 0:64])
                nc.vector.tensor_tensor(out=hl3[:, b, 64:128], in0=u_sb[:],
                                        in1=hif[:], op=ALU.subtract)
            nc.sync.dma_start(
                out=tab_o.rearrange("(b p) f -> p b f", p=128),
                in_=hl3)
    nc.compile()
    return nc


# ------------------------------------------------- shared gather-reduce loop
def emit_gather_reduce(nc, tc, sched, tab, idx_t, dst_t, iota_t, gp, Sp, pp,
                       rhs_cols, ps_tag, on_block):
    """Main loop: chunked dma_gather + one-hot matmul reduction.

    on_block(b, ps) is called right after block b's last accumulating matmul.
    """
    tile_blk = sched["tile_blk"]
    chunks = sched["chunks"]
    first, last = sched["first"], sched["last"]
    OB = 8

    ps_tiles = {}
    # S tiles built in batches of OB aligned to global tile index
    S_cur = {"tile": None, "base": -1}

    def get_S(t):
        base = (t // OB) * OB
        if S_cur["base"] != base:
            n = min(OB, sched["NT"] - base)
            S = Sp.tile([128, OB * 128], BF16, tag="S")
            nc.vector.tensor_tensor(
                out=S[:].rearrange("p (k d) -> p k d", d=128)[:, 0:n, :],
                in0=dst_t[:, base:base + n].to_broadcast([128, n, 128]),
                in1=iota_t[:].unsqueeze(1).broadcast_to([128, n, 128]),
                op=ALU.is_equal)
            S_cur["tile"] = S
            S_cur["base"] = base
        return S_cur["tile"][:, (t - S_cur["base"]) * 128:
                             (t - S_cur["base"] + 1) * 128]

    for ci, (q, t0, ntl) in enumerate(chunks):
        ch = ntl * 128
        gt = gp.tile([128, CHT * 128], BF16, tag=f"gt{ci % 6}", name="gt")
        gt3 = gt[:].rearrange("p (k f) -> p k f", f=128)
        nc.gpsimd.dma_gather(
            gt3[:, 0:ntl, :], tab[QROWS * q:QROWS * (q + 1), :],
            idx_t[:, t0 * 8:(t0 + ntl) * 8], ch, ch, 128,
            queue_num=ci % 4, single_packet=False)
        for tl in range(ntl):
            t = t0 + tl
            b = tile_blk[t]
            if b not in ps_tiles:
                ps_tiles[b] = pp.tile([128, rhs_cols], F32,
                                      tag=f"{ps_tag}{b % BGSZ}", space="PSUM",
                                      name=f"{ps_tag}{b % BGSZ}")
            S = get_S(t)
            nc.tensor.matmul(ps_tiles[b][:], lhsT=S,
                             rhs=gt3[:, tl, 0:rhs_cols],
                             start=(t == first[b]), stop=(t == last[b]))
            if t == last[b]:
                on_block(b, ps_tiles.pop(b))


# ---------------------------------------------------------------- L2
def build_L2(sched):
    nc = bacc.Bacc("TRN2", target_bir_lowering=False, debug=False,
                   num_devices=NC_N, num_swdge_queues=4)
    NT = sched["NT"]
    tab = nc.dram_tensor("tab", [N_ALL, 128], BF16, kind="ExternalInput").ap()
    idxs = nc.dram_tensor("idxs", [128, NT * 8], I16, kind="ExternalInput").ap()
    dlab = nc.dram_tensor("dlab", [128, NT], BF16, kind="ExternalInput").ap()
    iota = nc.dram_tensor("iota", [128, 128], BF16, kind="ExternalInput").ap()
    wc_rep = nc.dram_tensor("wc_rep", [128, 3 * 64], F32,
                            kind="ExternalInput").ap()
    u_own = nc.dram_tensor("u_own", [NPAD, 128], BF16, kind="ExternalInput").ap()
    sel_o = nc.dram_tensor("sel_o", [128, NBLK], F32, kind="ExternalOutput").ap()
    sumw_o = nc.dram_tensor("sumw_o", [128, NBLK], F32,
                            kind="ExternalOutput").ap()
    t2_o = nc.dram_tensor("t2_o", [NPAD, 64], BF16, kind="ExternalOutput").ap()

    with tile.TileContext(nc) as tc:
        with tc.tile_pool(name="const", bufs=1) as cp, \
             tc.tile_pool(name="sb", bufs=3) as sp, \
             tc.tile_pool(name="g", bufs=3) as gp, \
             tc.tile_pool(name="S", bufs=2) as Sp, \
             tc.tile_pool(name="ps", bufs=1, space="PSUM") as pp:
            nc.gpsimd.load_library(library_config.mlp)
            iota_t = cp.tile([128, 128], BF16)
            nc.sync.dma_start(out=iota_t[:], in_=iota[:])
            wc_t = cp.tile([128, 3 * 64], F32)
            nc.sync.dma_start(out=wc_t[:], in_=wc_rep[:])
            wc3 = wc_t[:].rearrange("p (j f) -> p j f", f=64)
            dst_t = cp.tile([128, NT], BF16)
            nc.sync.dma_start(out=dst_t[:], in_=dlab[:])
            idx_t = cp.tile([128, NT * 8], I16)
            nc.sync.dma_start(out=idx_t[:], in_=idxs[:])
            uo = cp.tile([128, NBLK * 128], BF16)
            uo3 = uo[:].rearrange("p (b f) -> p b f", f=128)
            nc.sync.dma_start(out=uo3,
                              in_=u_own.rearrange("(b p) f -> p b f", p=128))
            z_all = cp.tile([128, NBLK * 3], F32)
            z3 = z_all[:].rearrange("p (b j) -> p b j", j=3)

            def on_block(b, ps):
                rh = sp.tile([128, 64], F32, tag="rh")
                nc.scalar.copy(out=rh[:], in_=ps[:, 0:64])
                res = sp.tile([128, 64], F32, tag="res")
                nc.vector.tensor_add(res[:], rh[:], ps[:, 64:128])
                prod = sp.tile([128, 3 * 64], F32, tag="prod")
                nc.vector.tensor_tensor(
                    out=prod[:].rearrange("p (j f) -> p j f", f=64),
                    in0=res[:].unsqueeze(1).broadcast_to([128, 3, 64]),
                    in1=wc3, op=ALU.mult)
                nc.vector.tensor_reduce(
                    out=z3[:, b, :],
                    in_=prod[:].rearrange("p (j f) -> p j f", f=64),
                    axis=mybir.AxisListType.X, op=ALU.add)

            emit_gather_reduce(nc, tc, sched, tab, idx_t, dst_t, iota_t,
                               gp, Sp, pp, 128, "ps", on_block)

            # epilogue: sel / sumw / t2
            zr = cp.tile([128, NBLK * 2], F32)
            zr3 = zr[:].rearrange("p (b j) -> p b j", j=2)
            nc.vector.tensor_scalar_max(zr3[:, :, :], z3[:, :, 0:2], 0.0)
            dd = cp.tile([128, NBLK], F32)
            nc.vector.tensor_tensor(out=dd[:], in0=zr3[:, :, 1],
                                    in1=zr3[:, :, 0], op=ALU.subtract)
            sel = cp.tile([128, NBLK], F32)
            nc.vector.tensor_scalar(out=sel[:], in0=dd[:], scalar1=SEL_THR,
                                    scalar2=0.0, op0=ALU.is_gt)
            nc.sync.dma_start(out=sel_o[:], in_=sel[:])
            sumw = cp.tile([128, NBLK], F32)
            nc.vector.tensor_copy(out=sumw[:], in_=z3[:, :, 2])
            nc.sync.dma_start(out=sumw_o[:], in_=sumw[:])
            t2_all = cp.tile([128, NBLK * 64], BF16)
            t23 = t2_all[:].rearrange("p (b f) -> p b f", f=64)
            for b in range(NBLK):
                ub = sp.tile([128, 64], F32, tag="ub")
                nc.vector.tensor_add(ub[:], uo3[:, b, 0:64], uo3[:, b, 64:128])
                nc.vector.tensor_tensor(
                    out=t23[:, b, :], in0=ub[:],
                    in1=sel[:, b:b + 1].to_broadcast([128, 64]), op=ALU.mult)
            nc.sync.dma_start(
                out=t2_o.rearrange("(b p) f -> p b f", p=128),
                in_=t23)
    nc.compile()
    return nc


# ---------------------------------------------------------------- L3
def build_L3(sched):
    nc = bacc.Bacc("TRN2", target_bir_lowering=False, debug=False,
                   num_devices=NC_N, num_swdge_queues=4)
    NT = sched["NT"]
    tab = nc.dram_tensor("tab", [N_ALL, 128], BF16, kind="ExternalInput").ap()
    idxs = nc.dram_tensor("idxs", [128, NT * 8], I16, kind="ExternalInput").ap()
    dlab = nc.dram_tensor("dlab", [128, NT], BF16, kind="ExternalInput").ap()
    iota = nc.dram_tensor("iota", [128, 128], BF16, kind="ExternalInput").ap()
    wl_rep = nc.dram_tensor("wl_rep", [128, 64], F32, kind="ExternalInput").ap()
    u_own = nc.dram_tensor("u_own", [NPAD, 128], BF16, kind="ExternalInput").ap()
    sel_i = nc.dram_tensor("sel_i", [128, NBLK], F32, kind="ExternalInput").ap()
    sumw_i = nc.dram_tensor("sumw_i", [128, NBLK], F32,
                            kind="ExternalInput").ap()
    u3_o = nc.dram_tensor("u3_o", [NPAD, 64], BF16, kind="ExternalOutput").ap()

    with tile.TileContext(nc) as tc:
        with tc.tile_pool(name="const", bufs=1) as cp, \
             tc.tile_pool(name="sb", bufs=3) as sp, \
             tc.tile_pool(name="g", bufs=3) as gp, \
             tc.tile_pool(name="S", bufs=2) as Sp, \
             tc.tile_pool(name="ps", bufs=1, space="PSUM") as pp:
            nc.gpsimd.load_library(library_config.mlp)
            iota_t = cp.tile([128, 128], BF16)
            nc.sync.dma_start(out=iota_t[:], in_=iota[:])
            wl_t = cp.tile([128, 64], F32)
            nc.sync.dma_start(out=wl_t[:], in_=wl_rep[:])
            dst_t = cp.tile([128, NT], BF16)
            nc.sync.dma_start(out=dst_t[:], in_=dlab[:])
            idx_t = cp.tile([128, NT * 8], I16)
            nc.sync.dma_start(out=idx_t[:], in_=idxs[:])
            uo = cp.tile([128, NBLK * 128], BF16)
            uo3 = uo[:].rearrange("p (b f) -> p b f", f=128)
            nc.sync.dma_start(out=uo3,
                              in_=u_own.rearrange("(b p) f -> p b f", p=128))
            sel_t = cp.tile([128, NBLK], F32)
            nc.sync.dma_start(out=sel_t[:], in_=sel_i[:])
            sumw_t = cp.tile([128, NBLK], F32)
            nc.sync.dma_start(out=sumw_t[:], in_=sumw_i[:])
            s2_all = cp.tile([128, NBLK], F32)

            def on_block(b, ps):
                prod = sp.tile([128, 64], F32, tag="prod")
                nc.vector.tensor_tensor(out=prod[:], in0=ps[:], in1=wl_t[:],
                                        op=ALU.mult)
                nc.vector.tensor_reduce(
                    out=s2_all[:, b:b + 1], in_=prod[:],
                    axis=mybir.AxisListType.X, op=ALU.add)

            emit_gather_reduce(nc, tc, sched, tab, idx_t, dst_t, iota_t,
                               gp, Sp, pp, 64, "ps", on_block)

            zs = cp.tile([128, NBLK], F32)
            nc.vector.tensor_add(zs[:], s2_all[:], sumw_t[:])
            wsel = cp.tile([128, NBLK], F32)
            nc.scalar.activation(out=wsel[:], in_=zs[:], func=ACT.Sigmoid)
            g_all = cp.tile([128, NBLK], F32)
            nc.vector.tensor_tensor(out=g_all[:], in0=wsel[:], in1=sel_t[:],
                                    op=ALU.mult)
            u3_all = cp.tile([128, NBLK * 64], BF16)
            u33 = u3_all[:].rearrange("p (b f) -> p b f", f=64)
            for b in range(NBLK):
                ub = sp.tile([128, 64], F32, tag="ub")
                nc.vector.tensor_add(ub[:], uo3[:, b, 0:64], uo3[:, b, 64:128])
                nc.vector.tensor_tensor(
                    out=u33[:, b, :], in0=ub[:],
                    in1=g_all[:, b:b + 1].to_broadcast([128, 64]), op=ALU.mult)
            nc.sync.dma_start(
                out=u3_o.rearrange("(b p) f -> p b f", p=128),
                in_=u33)
    nc.compile()
    return nc


# ---------------------------------------------------------------- L4
def build_L4(sched):
    nc = bacc.Bacc("TRN2", target_bir_lowering=False, debug=False,
                   num_devices=NC_N, num_swdge_queues=4)
    NT = sched["NT"]
    tab = nc.dram_tensor("tab", [N_ALL, 128], BF16, kind="ExternalInput").ap()
    idxs = nc.dram_tensor("idxs", [128, NT * 8], I16, kind="ExternalInput").ap()
    dlab = nc.dram_tensor("dlab", [128, NT], BF16, kind="ExternalInput").ap()
    iota = nc.dram_tensor("iota", [128, 128], BF16, kind="ExternalInput").ap()
    u_own = nc.dram_tensor("u_own", [NPAD, 128], BF16, kind="ExternalInput").ap()
    out_o = nc.dram_tensor("out_o", [NPAD, 64], F32, kind="ExternalOutput").ap()

    with tile.TileContext(nc) as tc:
        with tc.tile_pool(name="const", bufs=1) as cp, \
             tc.tile_pool(name="sb", bufs=3) as sp, \
             tc.tile_pool(name="g", bufs=3) as gp, \
             tc.tile_pool(name="S", bufs=2) as Sp, \
             tc.tile_pool(name="ps", bufs=1, space="PSUM") as pp:
            nc.gpsimd.load_library(library_config.mlp)
            iota_t = cp.tile([128, 128], BF16)
            nc.sync.dma_start(out=iota_t[:], in_=iota[:])
            dst_t = cp.tile([128, NT], BF16)
            nc.sync.dma_start(out=dst_t[:], in_=dlab[:])
            idx_t = cp.tile([128, NT * 8], I16)
            nc.sync.dma_start(out=idx_t[:], in_=idxs[:])
            uo = cp.tile([128, NBLK * 128], BF16)
            uo3 = uo[:].rearrange("p (b f) -> p b f", f=128)
            nc.sync.dma_start(out=uo3,
                              in_=u_own.rearrange("(b p) f -> p b f", p=128))
            o_all = cp.tile([128, NBLK * 64], F32)
            o3 = o_all[:].rearrange("p (b f) -> p b f", f=64)
            n2_all = cp.tile([128, NBLK], F32)

            def on_block(b, ps):
                ax = sp.tile([128, 64], F32, tag="ax")
                nc.vector.tensor_scalar_max(ax[:], ps[:], 0.0)
                ub = sp.tile([128, 64], F32, tag="ub")
                nc.vector.tensor_add(ub[:], uo3[:, b, 0:64], uo3[:, b, 64:128])
                nc.vector.tensor_add(o3[:, b, :], ub[:], ax[:])
                sq = sp.tile([128, 64], F32, tag="sq")
                nc.scalar.activation(out=sq[:], in_=o3[:, b, :],
                                     func=ACT.Square,
                                     accum_out=n2_all[:, b:b + 1])

            emit_gather_reduce(nc, tc, sched, tab, idx_t, dst_t, iota_t,
                               gp, Sp, pp, 64, "ps", on_block)

            # expmap0 + proj factors
            nv = cp.tile([128, NBLK], F32)
            nc.scalar.activation(out=nv[:], in_=n2_all[:], func=ACT.Sqrt)
            nm = cp.tile([128, NBLK], F32)
            nc.vector.tensor_scalar_max(nm[:], nv[:], MIN_NORM)
            th = cp.tile([128, NBLK], F32)
            nc.scalar.activation(out=th[:], in_=nm[:], func=ACT.Tanh)
            rnm = cp.tile([128, NBLK], F32)
            nc.vector.reciprocal(rnm[:], nm[:])
            f1 = cp.tile([128, NBLK], F32)
            nc.vector.tensor_tensor(out=f1[:], in0=th[:], in1=rnm[:],
                                    op=ALU.mult)
            rt = cp.tile([128, NBLK], F32)
            nc.vector.reciprocal(rt[:], th[:])
            cap = cp.tile([128, NBLK], F32)
            nc.vector.tensor_scalar(out=cap[:], in0=rt[:], scalar1=PROJ_MAXN,
                                    scalar2=1.0, op0=ALU.mult, op1=ALU.min)
            f2 = cp.tile([128, NBLK], F32)
            nc.vector.tensor_tensor(out=f2[:], in0=f1[:], in1=cap[:],
                                    op=ALU.mult)
            oo_all = cp.tile([128, NBLK * 64], F32)
            oo3 = oo_all[:].rearrange("p (b f) -> p b f", f=64)
            for b in range(NBLK):
                nc.vector.tensor_tensor(
                    out=oo3[:, b, :], in0=o3[:, b, :],
                    in1=f2[:, b:b + 1].to_broadcast([128, 64]), op=ALU.mult)
            nc.sync.dma_start(
                out=out_o.rearrange("(b p) f -> p b f", p=128),
                in_=oo3)
    nc.compile()
    return nc


# ---------------------------------------------------------------- runner
def _run(nc, in_maps, trace):
    return bass_utils.run_bass_kernel_spmd(
        nc, in_maps, core_ids=list(range(NC_N)), trace=trace)


def kernel(x, edge_index, W_up, W_pl, W_lw, trace=None):
    if trace is None:
        trace = bool(int(os.environ.get("GNN_TRACE", "0")))

    x = np.asarray(x, np.float32)
    W_up = np.asarray(W_up, np.float32)
    W_pl = np.asarray(W_pl, np.float32)
    W_lw = np.asarray(W_lw, np.float32)
    cores, Klist, sched, slot_of = host_prep(edge_index)
    exec_times = []
    kernel.last_sched = sched

    iota = np.tile(np.arange(128, dtype=np.float32)[None, :],
                   (128, 1)).astype(BF)
    Wcat = np.concatenate([W_pl, W_lw[64:128]], axis=1)        # [64, 3]
    wc_rep = np.tile(Wcat.T.reshape(1, 3 * 64), (128, 1)).astype(np.float32)
    wl_rep = np.tile(W_lw[0:64, 0].reshape(1, 64), (128, 1)).astype(np.float32)

    # ---- L1
    x_pad = np.zeros((NC_N, NPAD, 128), np.float32)
    for c in range(NC_N):
        x_pad[c, slot_of[c, :NSH]] = x[c * NSH:(c + 1) * NSH]
    nc1 = build_L1()
    r1 = _run(nc1, [{"x": x_pad[c], "Wup": W_up} for c in range(NC_N)], trace)
    exec_times.append(r1.exec_time_ns)
    shards = [r1.results[c]["tab_o"] for c in range(NC_N)]

    # ---- L2
    tab2 = np.concatenate(shards, axis=0)                      # [N_ALL, 128]
    nc2 = build_L2(sched)
    r2 = _run(nc2, [{"tab": tab2, "idxs": cores[c]["idxw"],
                     "dlab": cores[c]["dstlab"], "iota": iota,
                     "wc_rep": wc_rep, "u_own": shards[c]}
                    for c in range(NC_N)], trace)
    exec_times.append(r2.exec_time_ns)
    sel = [r2.results[c]["sel_o"] for c in range(NC_N)]
    sumw = [r2.results[c]["sumw_o"] for c in range(NC_N)]
    t2 = [r2.results[c]["t2_o"] for c in range(NC_N)]

    # ---- L3
    tab3 = np.zeros((N_ALL, 128), BF)
    tab3[:, 0:64] = np.concatenate(t2, axis=0)
    nc3 = build_L3(sched)
    r3 = _run(nc3, [{"tab": tab3, "idxs": cores[c]["idxw"],
                     "dlab": cores[c]["dstlab"], "iota": iota,
                     "wl_rep": wl_rep, "u_own": shards[c],
                     "sel_i": sel[c], "sumw_i": sumw[c]}
                    for c in range(NC_N)], trace)
    exec_times.append(r3.exec_time_ns)
    u3 = [r3.results[c]["u3_o"] for c in range(NC_N)]

    # ---- L4
    tab4 = np.zeros((N_ALL, 128), BF)
    tab4[:, 0:64] = np.concatenate(u3, axis=0)
    nc4 = build_L4(sched)
    r4 = _run(nc4, [{"tab": tab4, "idxs": cores[c]["idxw"],
                     "dlab": cores[c]["dstlab"], "iota": iota,
                     "u_own": shards[c]}
                    for c in range(NC_N)], trace)
    exec_times.append(r4.exec_time_ns)
    out = np.concatenate(
        [r4.results[c]["out_o"][slot_of[c, :NSH]] for c in range(NC_N)],
        axis=0)

    kernel.last_exec_times = exec_times
    return out
